# revision 8
# baseline (speedup 1.0000x reference)
"""Ergodicity loss kernel for Trainium2 (8 NeuronCores, batch-sharded SPMD).

Math: loss = mean((c - coeffs)^2) + REG*sum(u^2)/(2*N*T*B)
      c[b,i,j] = sum_{t,n} cos(i*pi*x0)*cos(j*pi*x1) / (norm[i,j]*N*T)

Device computes, per core (4 of 32 batches; batch-sharded so no collective):
  - 16 "feature" tensors per spatial dim: fixed linear mixes of cos(k*pi*x_d)
    built from one ACT Sin (k=1), ACT Square chains (affine folded into the
    activation scale/bias) and DVE STT / tensor_mul.  Features are stored
    k-major CONTIGUOUS (2048 contiguous bf16 per feature per partition) so
    DVE tensor ops hit the 2x_1P perf mode (2 elem/cycle vs 1).
  - C'[b, k0*8+n, k1*8+n] via accumulating bf16 matmuls, operands addressed
    with 2-D free APs (k stride 2048, n stride 2); off-diagonal n-cells are
    junk and ignored on the host.
  - sum(u^2) on the PE: fp16 Gram-block self-matmuls into one psum tile
    (host sums the diagonal).  Inputs arrive as fp16 (host-cast), halving
    HBM traffic.
  - A junk-matmul stream at kernel start keeps the PE HAM activity monitor
    busy so the PE clock is at 2.4 GHz (not the cold 1.2 GHz) by the time
    the real matmul bursts issue.

Host recovers the true cos-basis C by inverting the feature-mixing matrix A
(replayed symbolically in a cos-harmonic algebra), then finishes in float64.

Toolchain notes: this walrus build enforces a 1-sync-wait budget on most
instruction templates: one sin per input-DMA chunk, an "opener" matmul per
slab that pre-observes the ACT semaphore on the PE so real matmuls carry at
most one DVE wait, GPSIMD memset order chosen so the first junk matmul's
single wait covers both memsets, and split kernel-tail drains.
"""

import sys

sys.path.insert(0, "/opt/trn_rl_repo")

import numpy as np

import concourse.bass as bass
import concourse.mybir as mybir
from concourse import bass_utils
from concourse.tile import TileContext
from concourse.tile_rust import add_dep_helper
from concourse.vector_clock import ScopedClock, VectorClock

# Tile's kernel-tail barrier emits ONE drain waiting on every live proc,
# which exceeds the per-instruction sync-wait budget.  Split it.
_orig_drain_and_barrier = TileContext._drain_and_barrier


def _split_drain_and_barrier(self, tick_clock, wait_clock):
    gc = tick_clock.global_clock
    ticks = list(gc)
    procs = [i for i, t in enumerate(ticks) if t > 0]
    for p in procs:
        vec = [0] * len(ticks)
        vec[p] = ticks[p]
        d = self.nc.sync.drain()
        wait_clock.add_sem_waits(d.ins, ScopedClock({None: VectorClock(vec)}))
    self.nc.all_engine_barrier(sem_only=True)
    popped = self.nc._tile_sem_poison_stack.pop()
    assert popped is self._sem_poison
    self.nc.clear_and_free_semaphores(list(self.sems.allocated().values()))
    self.nc.all_engine_barrier(sem_only=True)


TileContext._drain_and_barrier = _split_drain_and_barrier

# Problem constants (hardcoded per spec).
K_MAX = 16
N_AGENTS = 64
T = 512
B = 32
D = 2
REG = 1e-3
N_CORES = 8
BPC = B // N_CORES  # batches per core = 4

PI = float(np.pi)

F32 = mybir.dt.float32
F16 = mybir.dt.float16
BF16 = mybir.dt.bfloat16

# Per-core geometry: x shard [T=512, BPC=4, N=64, D=2] is host-permuted to
# [128, 2048] fp16 with partition p = t % 128 and column
# tc*512 + b*128 + n*2 + d  (tc = t // 128).
TC = 4
COLS = TC * BPC * N_AGENTS * D  # 2048
CHUNK = COLS // 4  # 512 cols per input-DMA chunk (= one tc)
NPROBE = 16

# PE warmup stream sizes (junk matmuls keeping the HAM activity monitor hot).
J1 = 30  # before the u-gram
J2 = 24  # between u-gram and the slab-0 feature matmuls


# ---------------------------------------------------------------------------
# Symbolic harmonic algebra: every feature is a fixed linear combination of
# cos(k*pi*x), k = 0..15.  Replay the device pipeline to get the mixing
# matrix A (features x harmonics); the host inverts it exactly.
# ---------------------------------------------------------------------------
class Harm:
    __slots__ = ("c",)

    def __init__(self, c):
        self.c = np.asarray(c, dtype=np.float64)

    @staticmethod
    def const(v):
        c = np.zeros(K_MAX)
        c[0] = v
        return Harm(c)

    @staticmethod
    def basis(k, v=1.0):
        c = np.zeros(K_MAX)
        c[k] = v
        return Harm(c)

    def affine(self, scale, bias):
        c = self.c * scale
        c[0] += bias
        return Harm(c)

    def mul(self, other):
        out = np.zeros(K_MAX)
        for a in range(K_MAX):
            if self.c[a] == 0.0:
                continue
            for b in range(K_MAX):
                if other.c[b] == 0.0:
                    continue
                v = self.c[a] * other.c[b]
                s, d = a + b, abs(a - b)
                assert s < K_MAX or v == 0.0, f"harmonic overflow {a}+{b}"
                out[s] += 0.5 * v
                out[d] += 0.5 * v
        return Harm(out)

    def square(self, scale=1.0, bias=0.0):
        z = self.affine(scale, bias)
        return z.mul(z)

    def stt(self, s, other):  # (self - s) * other
        return self.affine(1.0, -s).mul(other)


def _feature_mixing_matrix():
    """Replay the device feature pipeline symbolically -> A[16,16].
    Must mirror the ops in _body exactly."""
    f = [None] * K_MAX
    f[0] = Harm.const(1.0)
    f[1] = Harm.basis(1, -1.0)  # Sin(pi*x - pi/2) = -cos(pi*x)
    f[2] = f[1].square()  # ACT: (c2+1)/2
    f[4] = f[2].square(2.0, -1.0)  # ACT: (c4+1)/2
    f[8] = f[4].square(2.0, -1.0)  # ACT: (c8+1)/2
    f[3] = f[2].stt(0.75, f[1])  # DVE STT: -c3/4
    f[6] = f[3].square(4.0, 0.0)  # ACT: (c6+1)/2
    f[5] = f[4].stt(0.5, f[1])  # DVE STT
    f[7] = f[6].stt(0.5, f[1])  # DVE STT
    f[12] = f[6].stt(0.5, f[6])  # DVE STT
    f[9] = f[8].mul(f[1])  # DVE TT
    f[10] = f[5].mul(f[5])  # DVE TT
    f[14] = f[7].mul(f[7])  # DVE TT
    f[15] = f[14].mul(f[1])  # DVE TT
    f[11] = f[10].mul(f[1])  # DVE TT
    f[13] = f[12].mul(f[1])  # DVE TT
    return np.stack([x.c for x in f])


_A = _feature_mixing_matrix()
_AINV = np.linalg.inv(_A)
assert np.linalg.cond(_A) < 1e4, np.linalg.cond(_A)


def _np_constants():
    """numpy copy of reference._constants() for L=(1,1)."""
    ks = np.arange(K_MAX, dtype=np.float64)
    vs = []
    for _ in range(D):
        with np.errstate(divide="ignore", invalid="ignore"):
            ki = ks * np.pi
            nz = (np.exp(1j * ki) - 1.0) / (1j * ki)
        integral = np.where(ks == 0, 1.0 + 0j, nz)
        vs.append(integral)
    cd = np.real(vs[0][:, None] * vs[1][None, :]).astype(np.float64)
    norm_last = np.where(ks == 0, 1.0, np.sqrt(0.5))
    norm = np.broadcast_to(norm_last[None, :], (K_MAX, K_MAX)).copy()
    return cd / norm, norm


_COEFFS, _NORM = _np_constants()


# ---------------------------------------------------------------------------
# Device program
# ---------------------------------------------------------------------------
def _body(nc, tc, xu_in, pr_in, out_dram):
    Sq = mybir.ActivationFunctionType.Square
    Sin = mybir.ActivationFunctionType.Sin
    sub = mybir.AluOpType.subtract
    mult = mybir.AluOpType.mult

    with (
        tc.tile_pool(name="io", bufs=1) as io_pool,
        tc.tile_pool(name="feat", bufs=1) as feat_pool,
        tc.tile_pool(name="work", bufs=1) as work_pool,
        tc.tile_pool(name="psum", bufs=1, space="PSUM") as psum_pool,
    ):
        xu = io_pool.tile([128, 2 * COLS], F16, tag="xu")
        pr = io_pool.tile([128, NPROBE], F32, tag="pr")
        # x: 4 chunk DMAs (one per tc); u: 2 halves; probe: 1. (+1 out = 8
        # HW queue sems, each DMA on its own proc => <=1 sync wait each.)
        for ci in range(4):
            nc.sync.dma_start(
                out=xu[:, ci * CHUNK : (ci + 1) * CHUNK],
                in_=xu_in[:, ci * CHUNK : (ci + 1) * CHUNK],
            )
        for ci in range(2):
            a = COLS + ci * (COLS // 2)
            b = COLS + (ci + 1) * (COLS // 2)
            nc.sync.dma_start(out=xu[:, a:b], in_=xu_in[:, a:b])
        nc.sync.dma_start(out=pr[:], in_=pr_in[:])

        uraw = xu[:, COLS : 2 * COLS]

        # --- SBUF feature storage: block layout.  Column index is
        # blk*256 + k*16 + nl*2 + d with blk = (tc*4+b)*8+oc (0..127).
        # Feature ops see [[256, nblk], [1, 16]] APs (16-elem unit-stride
        # inner runs for the DVE 2x_1P mode); matmul operands for
        # (blk, d) are single-stride [[2, 128]] over kn = k*8+nl (the
        # walrus verifier allows only ONE free dim on the moving AP).
        NBLK = 128
        FA = feat_pool.tile([128, K_MAX * COLS], BF16, tag="FA")
        FAke = FA[:].rearrange(
            "p (blk k e) -> p k blk e", blk=NBLK, k=K_MAX, e=16
        )
        FAm = FA[:].rearrange(
            "p (blk kn d) -> p blk d kn", blk=NBLK, kn=128, d=D
        )

        def F(k, b0=0, b1=NBLK):
            return FAke[:, k, b0:b1]

        warm = work_pool.tile([128, 128], BF16, tag="warm")
        csb = work_pool.tile([128, BPC * 128 + 128 + NPROBE], F32, tag="csb")

        pstiles = [
            psum_pool.tile([128, 128], F32, tag=f"ps{b}", name=f"ps{b}")
            for b in range(BPC)
        ]
        psu = psum_pool.tile([128, 128], F32, tag="psu")
        pjunk = psum_pool.tile([128, 128], F32, tag="pjunk")

        # Tile's scheduler may reorder within an engine stream; that breaks
        # the opener trick (which assumes the opener's read tick covers all
        # earlier feature writes).  Pin each engine's program order with
        # sync-free dep edges.
        _last = {}

        def _pin(key, bi):
            if key in _last:
                add_dep_helper(bi.ins, _last[key].ins, sync=False,
                               reason=f"{key} order pin")
            _last[key] = bi
            return bi

        # --- GPSIMD: f0 = ones first, warm tile second; the first junk
        # matmul's single gpsimd wait then covers both. ---
        _pin("gp", nc.gpsimd.memset(F(0), 1.0))
        _pin("gp", nc.gpsimd.memset(warm[:], 1.0))

        # --- ACT stream ---
        # Probe: one Sin over host-precomputed args (LUT range diagnostic).
        _pin("act", nc.scalar.activation(
            csb[:, BPC * 128 + 128 :], pr[:], Sin, scale=1.0
        ))

        HB = NBLK // 2  # blocks per slab
        SL = {0: (0, HB), 1: (HB, NBLK)}

        def act(out, in_, func, **kw):
            _pin("act", nc.scalar.activation(out, in_, func, **kw))

        def sin_chunk(ci):
            a, b = ci * CHUNK, (ci + 1) * CHUNK
            act(F(1, ci * 32, (ci + 1) * 32), xu[:, a:b], Sin,
                scale=PI, bias=-PI / 2)

        def stt(k_out, k_in, s, k_mul, sl):
            b0, b1 = sl
            _pin("dve", nc.vector.scalar_tensor_tensor(
                F(k_out, b0, b1), F(k_in, b0, b1), s, F(k_mul, b0, b1),
                sub, mult,
            ))

        def tt(k_out, k_a, k_b, sl):
            b0, b1 = sl
            _pin("dve", nc.vector.tensor_mul(
                out=F(k_out, b0, b1), in0=F(k_a, b0, b1), in1=F(k_b, b0, b1)
            ))

        # ACT order: sins early; f2 of a slab as soon as its sins are done;
        # f6 waits on DVE's f3.  Interleaved manually with the DVE stream
        # below purely via program order + Tile semaphores.
        sin_chunk(0)
        sin_chunk(1)

        # slab 0 ACT: f2 then (after DVE f3) f6, f4, f8
        act(F(2, *SL[0]), F(1, *SL[0]), Sq)
        sin_chunk(2)
        sin_chunk(3)

        # --- DVE stream (slab 0 head) ---
        stt(3, 2, 0.75, 1, SL[0])

        act(F(6, *SL[0]), F(3, *SL[0]), Sq, scale=4.0)
        act(F(4, *SL[0]), F(2, *SL[0]), Sq, scale=2.0, bias=-1.0)
        act(F(8, *SL[0]), F(4, *SL[0]), Sq, scale=2.0, bias=-1.0)

        stt(7, 6, 0.5, 1, SL[0])
        stt(12, 6, 0.5, 6, SL[0])
        stt(5, 4, 0.5, 1, SL[0])
        tt(14, 7, 7, SL[0])
        tt(10, 5, 5, SL[0])
        tt(9, 8, 1, SL[0])

        # slab 1 ACT chain (f2 early so DVE's f3 s1 can go; f6 s1 waits it)
        act(F(2, *SL[1]), F(1, *SL[1]), Sq)
        stt(3, 2, 0.75, 1, SL[1])
        act(F(6, *SL[1]), F(3, *SL[1]), Sq, scale=4.0)
        act(F(4, *SL[1]), F(2, *SL[1]), Sq, scale=2.0, bias=-1.0)
        act(F(8, *SL[1]), F(4, *SL[1]), Sq, scale=2.0, bias=-1.0)

        # slab 0 leaves, split per tc so the tc0 matmul burst can start
        # before tc1's features are done.
        for tcc in (0, 1):
            hs = (tcc * 32, (tcc + 1) * 32)
            tt(15, 14, 1, hs)
            tt(11, 10, 1, hs)
            tt(13, 12, 1, hs)

        # slab 1 DVE tail
        stt(7, 6, 0.5, 1, SL[1])
        stt(12, 6, 0.5, 6, SL[1])
        stt(5, 4, 0.5, 1, SL[1])
        tt(14, 7, 7, SL[1])
        tt(10, 5, 5, SL[1])
        tt(9, 8, 1, SL[1])
        for tcc in (2, 3):
            hs = (tcc * 32, (tcc + 1) * 32)
            tt(15, 14, 1, hs)
            tt(11, 10, 1, hs)
            tt(13, 12, 1, hs)

        # --- PE stream ---
        # J1 junk matmuls (HAM warmup), u-gram, J2 junk, then the real
        # bursts.  Junk matmuls read the gpsimd-initialized warm tile; the
        # first one's single gpsimd wait also covers the f0 memset.
        last_mm = None
        for j in range(J1):
            mm = nc.tensor.matmul(
                pjunk[:], warm[:], warm[:], start=True, stop=True,
                skip_group_check=True,
            )
            if last_mm is not None:
                add_dep_helper(mm.ins, last_mm.ins, sync=False,
                               reason="junk chain")
            last_mm = mm

        for c in range(16):
            ub = uraw[:, c * 128 : (c + 1) * 128]
            mm = nc.tensor.matmul(
                psu[:], ub, ub, start=(c == 0), stop=(c == 15),
                skip_group_check=True,
            )
            add_dep_helper(mm.ins, last_mm.ins, sync=False, reason="ugram")
            last_mm = mm

        for j in range(J2):
            mm = nc.tensor.matmul(
                pjunk[:], warm[:], warm[:], start=True, stop=True,
                skip_group_check=True,
            )
            add_dep_helper(mm.ins, last_mm.ins, sync=False,
                           reason="junk chain 2")
            last_mm = mm

        # Real bursts: per slab an opener matmul reads the slab's LAST
        # ACT-written feature (f8) so every real Ldweights carries at most
        # the single DVE wait its template allows.
        for si in (0, 1):
            b0, b1 = SL[si]
            fstub = FAke[:, 8, b1 - 1 : b1, 15:16]
            op = nc.tensor.matmul(
                pjunk[0:1, 120 + si : 121 + si],
                fstub, fstub,
                start=True, stop=True, skip_group_check=True,
            )
            add_dep_helper(op.ins, last_mm.ins, sync=False,
                           reason="opener after prev MMs")
            opener = op
            last_mm = op
            for tcc in (2 * si, 2 * si + 1):
                for b in range(BPC):
                    ps = pstiles[b]
                    for oc in range(8):
                        blk = (tcc * BPC + b) * 8 + oc
                        mm = nc.tensor.matmul(
                            ps[:],
                            FAm[:, blk, 0],
                            FAm[:, blk, 1],
                            start=(tcc == 0 and oc == 0),
                            stop=(tcc == TC - 1 and oc == 7),
                            skip_group_check=True,
                        )
                        add_dep_helper(mm.ins, opener.ins, sync=False,
                                       reason="PE wait-slot opener")
                        last_mm = mm

        # PSUM -> SBUF on ACT; single-engine csb producers keep the output
        # DMA at one sync wait.
        for b in range(BPC):
            _pin("act", nc.scalar.copy(
                out=csb[:, b * 128 : (b + 1) * 128], in_=pstiles[b][:]))
        _pin("act", nc.scalar.copy(
            out=csb[:, BPC * 128 : BPC * 128 + 128], in_=psu[:]))
        nc.sync.dma_start(out=out_dram[:], in_=csb[:])


_CACHE = {}


def _register_const(nc, value, dtype=F32):
    t = nc.alloc_sbuf_tensor(f"const-{dtype.name}-{value}", [128, 1], dtype)
    nc.gpsimd.memset(t.ap(), value)
    nc.const_aps.aps[(dtype, value)] = t.ap()


def _build():
    if "nc" in _CACHE:
        return _CACHE["nc"]
    nc = bass.Bass("TRN2", debug=False)
    # Skip the kernel-tail DGE-queue drain (~3-4us); all DMAs are already
    # completion-waited by the split drains and no dynamic DMA state is
    # used.  Second-execution correctness is validated by the harness.
    type(nc.gpsimd).dma_reset = lambda self, semaphore_range=None: None
    _register_const(nc, -PI / 2)
    _register_const(nc, -1.0)
    nc.all_engine_barrier()
    xu_in = nc.dram_tensor("xu", [128, 2 * COLS], F16, kind="ExternalInput")
    pr_in = nc.dram_tensor("pr", [128, NPROBE], F32, kind="ExternalInput")
    out_d = nc.dram_tensor(
        "out", [128, BPC * 128 + 128 + NPROBE], F32, kind="ExternalOutput"
    )
    with TileContext(nc) as t:
        _body(nc, t, xu_in.ap(), pr_in.ap(), out_d.ap())
    _CACHE["nc"] = nc
    return nc


def _shard_host(a):
    """[T, B, N, D] -> per-core [128, COLS] fp16, col = tc*512+b*128+n*2+d."""
    out = []
    for c in range(N_CORES):
        s = a[:, c * BPC : (c + 1) * BPC]  # [512, 4, 64, 2]
        s = s.reshape(TC, 128, BPC, N_AGENTS, D)  # (tc, p, b, n, d)
        s = np.ascontiguousarray(np.transpose(s, (1, 0, 2, 3, 4)))
        out.append(s.reshape(128, COLS).astype(np.float16))
    return out


def _probe_args():
    xp = (np.arange(128, dtype=np.float64) + 0.5) / 128.0
    ks = np.arange(NPROBE, dtype=np.float64)
    return (ks[None, :] * np.pi * xp[:, None] - np.pi / 2).astype(np.float32)


def _make_inmaps(x, u):
    xs = _shard_host(np.asarray(x, dtype=np.float32))
    us = _shard_host(np.asarray(u, dtype=np.float32))
    prb = _probe_args()
    return [
        {
            "xu": np.ascontiguousarray(np.concatenate([xs[c], us[c]], axis=1)),
            "pr": prb,
        }
        for c in range(N_CORES)
    ]


def kernel(x, u, **_):
    nc = _build()
    in_maps = _make_inmaps(x, u)
    res = bass_utils.run_bass_kernel_spmd(nc, in_maps, core_ids=list(range(N_CORES)))
    return _finish_host(res.results)


_LAST_PROBE = None


def _finish_host(outs):
    """Host reduction/unmixing in float64 -> scalar loss."""
    global _LAST_PROBE
    Cp = np.zeros((B, K_MAX, K_MAX), dtype=np.float64)
    u2 = 0.0
    for c in range(N_CORES):
        o = outs[c]["out"].astype(np.float64)  # [128, 656]
        craw = o[:, : BPC * 128]
        ublk = o[:, BPC * 128 : BPC * 128 + 128]
        u2 += float(np.trace(ublk))
        for b in range(BPC):
            blk = craw[:, b * 128 : (b + 1) * 128]
            acc = np.zeros((K_MAX, K_MAX))
            for nl in range(8):
                acc += blk[nl::8, nl::8]
            Cp[c * BPC + b] = acc
    # Probe diagnostic: device Sin(arg) vs true sin(arg), per harmonic k.
    o0 = outs[0]["out"].astype(np.float64)
    dev = o0[:, BPC * 128 + 128 :]
    tru = np.sin(_probe_args().astype(np.float64))
    _LAST_PROBE = np.max(np.abs(dev - tru), axis=0)

    # C' = A C_true A^T -> C_true = Ainv C' Ainv^T
    Ct = np.einsum("ik,bkl,jl->bij", _AINV, Cp, _AINV)
    c = Ct / (_NORM[None] * (N_AGENTS * T))
    loss = np.mean((c - _COEFFS[None]) ** 2)
    loss = loss + REG * u2 / (2.0 * N_AGENTS * T * B)
    return np.array(loss, dtype=np.float32)


if __name__ == "__main__":
    rng = np.random.default_rng(0)
    x = rng.random((T, B, N_AGENTS, D), dtype=np.float32)
    u = rng.standard_normal((T, B, N_AGENTS, D)).astype(np.float32)
    print(kernel(x=x, u=u))
    print("probe err per k:", _LAST_PROBE)


# revision 10
# speedup vs baseline: 1.2728x; 1.2728x over previous
"""Ergodicity loss kernel for Trainium2 (8 NeuronCores, batch-sharded SPMD).

Math: loss = mean((c - coeffs)^2) + REG*sum(u^2)/(2*N*T*B)
      c[b,i,j] = sum_{t,n} cos(i*pi*x0)*cos(j*pi*x1) / (norm[i,j]*N*T)

Device computes, per core (4 of 32 batches; batch-sharded so no collective):
  - 16 "feature" tensors per spatial dim: fixed linear mixes of cos(k*pi*x_d)
    built from one ACT Sin (k=1), ACT Square ops (affine folded into the
    activation scale/bias; evens f2,f4,f8,f10,f12) and DVE ops (one STT for
    the clean -c3/4, plain tensor_muls for the rest).  Conditioning of the
    mixing matrix is checked row-normalized (device errors are relative).
  - Feature storage is a block layout: column = blk*256 + k*16 + nl*2 + d
    with blk = (tc*4+b)*8+oc, so matmul operands are single-free-dim
    [[2,128]] APs (the walrus verifier requires that) while feature ops see
    [[256,n],[1,16]] APs (16-elem unit-stride runs for DVE 2x_1P).
  - C'[b, k0*8+n, k1*8+n] via accumulating bf16 matmuls; off-diagonal
    n-cells are junk, dropped on the host.
  - sum(u^2) on the PE: fp16 Gram-block self-matmuls into one psum tile
    (host sums the diagonal).  Inputs arrive as fp16 (host-cast), halving
    HBM traffic; x comes as two 2 KB-per-partition-row chunks (1 KB rows
    halve effective DMA bandwidth).
  - A junk-matmul stream at kernel start keeps the PE HAM activity monitor
    busy so the PE clock is at 2.4 GHz for the u-gram / first bursts.

The Sin activation LUT is only accurate for |arg| <= ~pi/2 (measured via
the probe), hence exactly one Sin per x element and product chains for all
higher harmonics.

Host recovers the true cos-basis C by inverting the feature-mixing matrix A
(replayed symbolically in a cos-harmonic algebra), then finishes in float64.

Toolchain notes: this walrus build enforces a 1-sync-wait budget on most
instruction templates.  Structural consequences: engine-internal program
order is pinned with sync-free dep edges (Tile's scheduler otherwise
reorders and breaks the opener trick), an "opener" matmul per slab
pre-observes the ACT semaphore on the PE so real matmuls carry only the
one DVE wait their template allows, activation bias constants ride in the
"pr" input DMA (observed once by the first ACT op) instead of pre-barrier
gpsimd memsets, and the kernel-tail barrier is split into per-proc drains.
"""

import sys

sys.path.insert(0, "/opt/trn_rl_repo")

import numpy as np

import concourse.bass as bass
import concourse.mybir as mybir
from concourse import bass_utils
from concourse.tile import TileContext
from concourse.tile_rust import add_dep_helper
from concourse.vector_clock import ScopedClock, VectorClock

_orig_drain_and_barrier = TileContext._drain_and_barrier


def _split_drain_and_barrier(self, tick_clock, wait_clock):
    gc = tick_clock.global_clock
    ticks = list(gc)
    procs = [i for i, t in enumerate(ticks) if t > 0]
    for p in procs:
        vec = [0] * len(ticks)
        vec[p] = ticks[p]
        d = self.nc.sync.drain()
        wait_clock.add_sem_waits(d.ins, ScopedClock({None: VectorClock(vec)}))
    self.nc.all_engine_barrier(sem_only=True)
    popped = self.nc._tile_sem_poison_stack.pop()
    assert popped is self._sem_poison
    self.nc.clear_and_free_semaphores(list(self.sems.allocated().values()))
    self.nc.all_engine_barrier(sem_only=True)


TileContext._drain_and_barrier = _split_drain_and_barrier

# Problem constants (hardcoded per spec).
K_MAX = 16
N_AGENTS = 64
T = 512
B = 32
D = 2
REG = 1e-3
N_CORES = 8
BPC = B // N_CORES  # batches per core = 4

PI = float(np.pi)

F32 = mybir.dt.float32
F16 = mybir.dt.float16
BF16 = mybir.dt.bfloat16

# Per-core geometry: x shard [T=512, BPC=4, N=64, D=2] is host-permuted to
# [128, 2048] fp16 with partition p = t % 128 and column
# tc*512 + b*128 + n*2 + d  (tc = t // 128).
TC = 4
COLS = TC * BPC * N_AGENTS * D  # 2048
NBLK = 128  # feature blocks (tc, b, oc)
NPROBE = 16
NPR = NPROBE + 2  # probe cols + {-pi/2, -1.0} activation-bias consts

# PE warmup stream sizes (junk matmuls keeping the HAM activity monitor hot).
J1 = 24
J2 = 16


# ---------------------------------------------------------------------------
# Symbolic harmonic algebra.
# ---------------------------------------------------------------------------
class Harm:
    __slots__ = ("c",)

    def __init__(self, c):
        self.c = np.asarray(c, dtype=np.float64)

    @staticmethod
    def const(v):
        c = np.zeros(K_MAX)
        c[0] = v
        return Harm(c)

    @staticmethod
    def basis(k, v=1.0):
        c = np.zeros(K_MAX)
        c[k] = v
        return Harm(c)

    def affine(self, scale, bias):
        c = self.c * scale
        c[0] += bias
        return Harm(c)

    def mul(self, other):
        out = np.zeros(K_MAX)
        for a in range(K_MAX):
            if self.c[a] == 0.0:
                continue
            for b in range(K_MAX):
                if other.c[b] == 0.0:
                    continue
                v = self.c[a] * other.c[b]
                s, d = a + b, abs(a - b)
                assert s < K_MAX or v == 0.0, f"harmonic overflow {a}+{b}"
                out[s] += 0.5 * v
                out[d] += 0.5 * v
        return Harm(out)

    def square(self, scale=1.0, bias=0.0):
        z = self.affine(scale, bias)
        return z.mul(z)

    def stt(self, s, other):  # (self - s) * other
        return self.affine(1.0, -s).mul(other)


def _feature_mixing_matrix():
    """Replay the device feature pipeline symbolically -> A[16,16].
    Must mirror the ops in _body exactly."""
    f = [None] * K_MAX
    f[0] = Harm.const(1.0)
    f[1] = Harm.basis(1, -1.0)      # ACT Sin(pi*x - pi/2) = -cos(pi*x)
    f[2] = f[1].square()             # ACT
    f[4] = f[2].square(2.0, -1.0)    # ACT
    f[8] = f[4].square(2.0, -1.0)    # ACT
    f[3] = f[2].stt(0.75, f[1])      # DVE STT -> -c3/4
    f[6] = f[3].mul(f[3])            # DVE TT -> (c6+1)/32
    f[5] = f[4].mul(f[1])            # DVE TT
    f[7] = f[6].mul(f[1])            # DVE TT
    f[10] = f[5].square(4.0, 0.0)    # ACT
    f[12] = f[6].square(32.0, -1.0)  # ACT -> (c12+1)/2
    f[9] = f[8].mul(f[1])            # DVE TT
    f[14] = f[7].mul(f[7])           # DVE TT
    f[11] = f[10].mul(f[1])          # DVE leaf
    f[13] = f[12].mul(f[1])          # DVE leaf
    f[15] = f[14].mul(f[1])          # DVE leaf
    return np.stack([x.c for x in f])


_A = _feature_mixing_matrix()
_AINV = np.linalg.inv(_A)
_ROWCOND = np.linalg.cond(_A / np.linalg.norm(_A, axis=1, keepdims=True))
assert _ROWCOND < 1e3, _ROWCOND


def _np_constants():
    ks = np.arange(K_MAX, dtype=np.float64)
    vs = []
    for _ in range(D):
        with np.errstate(divide="ignore", invalid="ignore"):
            ki = ks * np.pi
            nz = (np.exp(1j * ki) - 1.0) / (1j * ki)
        integral = np.where(ks == 0, 1.0 + 0j, nz)
        vs.append(integral)
    cd = np.real(vs[0][:, None] * vs[1][None, :]).astype(np.float64)
    norm_last = np.where(ks == 0, 1.0, np.sqrt(0.5))
    norm = np.broadcast_to(norm_last[None, :], (K_MAX, K_MAX)).copy()
    return cd / norm, norm


_COEFFS, _NORM = _np_constants()


# ---------------------------------------------------------------------------
# Device program
# ---------------------------------------------------------------------------
def _body(nc, tc, xu_in, pr_in, out_dram):
    Sq = mybir.ActivationFunctionType.Square
    Sin = mybir.ActivationFunctionType.Sin
    sub = mybir.AluOpType.subtract
    mult = mybir.AluOpType.mult

    with (
        tc.tile_pool(name="io", bufs=1) as io_pool,
        tc.tile_pool(name="feat", bufs=1) as feat_pool,
        tc.tile_pool(name="work", bufs=1) as work_pool,
        tc.tile_pool(name="psum", bufs=1, space="PSUM") as psum_pool,
    ):
        xu = io_pool.tile([128, 2 * COLS], F16, tag="xu")
        pr = io_pool.tile([128, NPR], F32, tag="pr")
        # Activation bias consts ride in the pr DMA; the probe Sin (first
        # ACT op) observes that queue once for the whole ACT stream.
        nc.const_aps.aps[(F32, -PI / 2)] = pr[:, NPROBE : NPROBE + 1]
        nc.const_aps.aps[(F32, -1.0)] = pr[:, NPROBE + 1 : NPROBE + 2]

        # DMA kicks (each dma_start = one HW queue): pr first (consts),
        # then the two x slabs (2 KB rows), then u (4 KB rows).
        nc.sync.dma_start(out=pr[:], in_=pr_in[:])
        HC = COLS // 2
        nc.sync.dma_start(out=xu[:, 0:HC], in_=xu_in[:, 0:HC])
        nc.sync.dma_start(out=xu[:, HC:COLS], in_=xu_in[:, HC:COLS])
        nc.sync.dma_start(out=xu[:, COLS:], in_=xu_in[:, COLS:])
        uraw = xu[:, COLS : 2 * COLS]

        # Feature storage block layout (see module docstring).
        FA = feat_pool.tile([128, K_MAX * COLS], BF16, tag="FA")
        FAke = FA[:].rearrange(
            "p (blk k e) -> p k blk e", blk=NBLK, k=K_MAX, e=16
        )
        FAm = FA[:].rearrange(
            "p (blk kn d) -> p blk d kn", blk=NBLK, kn=128, d=D
        )

        def F(k, b0=0, b1=NBLK):
            return FAke[:, k, b0:b1]

        warm = work_pool.tile([128, 128], BF16, tag="warm")
        csb = work_pool.tile([128, BPC * 128 + 128 + NPROBE], F32, tag="csb")

        pstiles = [
            psum_pool.tile([128, 128], F32, tag=f"ps{b}", name=f"ps{b}")
            for b in range(BPC)
        ]
        psu = psum_pool.tile([128, 128], F32, tag="psu")
        pjunk = psum_pool.tile([128, 128], F32, tag="pjunk")

        # Pin engine-internal program order (Tile reorders otherwise).
        _last = {}

        def _pin(key, bi):
            if key in _last:
                add_dep_helper(bi.ins, _last[key].ins, sync=False,
                               reason=f"{key} order pin")
            _last[key] = bi
            return bi

        # GPSIMD: f0 = ones first, warm second; the first junk matmul's
        # single gpsimd wait covers both.
        _pin("gp", nc.gpsimd.memset(F(0), 1.0))
        _pin("gp", nc.gpsimd.memset(warm[:], 1.0))

        HB = NBLK // 2
        SL = {0: (0, HB), 1: (HB, NBLK)}

        def act(out, in_, func, **kw):
            return _pin("act", nc.scalar.activation(out, in_, func, **kw))

        def stt(k_out, k_in, s, k_mul, sl):
            b0, b1 = sl
            return _pin("dve", nc.vector.scalar_tensor_tensor(
                F(k_out, b0, b1), F(k_in, b0, b1), s, F(k_mul, b0, b1),
                sub, mult,
            ))

        def tt(k_out, k_a, k_b, sl):
            b0, b1 = sl
            return _pin("dve", nc.vector.tensor_mul(
                out=F(k_out, b0, b1), in0=F(k_a, b0, b1), in1=F(k_b, b0, b1)
            ))

        def sq(k_out, k_in, sl, scale=1.0, bias=0.0):
            b0, b1 = sl
            return act(F(k_out, b0, b1), F(k_in, b0, b1), Sq,
                       scale=scale, bias=bias)

        # --- ACT + DVE streams ---
        # Issue order MUST be topological (producer before consumer): Tile
        # tracks dataflow by issue order; a consumer issued before its
        # producer reads "uninitialized" bytes and the real write becomes a
        # reversed WAR anti-dep.  Per-engine execution order = issue order
        # (pinned above).
        act(csb[:, BPC * 128 + 128 :], pr[:, :NPROBE], Sin, scale=1.0)  # probe
        act(F(1, 0, HB), xu[:, 0:HC], Sin, scale=PI, bias=-PI / 2)      # sin s0
        act(F(1, HB, NBLK), xu[:, HC:COLS], Sin, scale=PI, bias=-PI / 2)
        sq(2, 1, SL[0])
        sq(2, 1, SL[1])
        stt(3, 2, 0.75, 1, SL[0])
        stt(3, 2, 0.75, 1, SL[1])
        sq(4, 2, SL[0], 2.0, -1.0)
        sq(4, 2, SL[1], 2.0, -1.0)
        tt(6, 3, 3, SL[0])
        tt(6, 3, 3, SL[1])
        sq(8, 4, SL[0], 2.0, -1.0)
        sq(8, 4, SL[1], 2.0, -1.0)
        tt(7, 6, 1, SL[0])
        tt(5, 4, 1, SL[0])
        tt(14, 7, 7, SL[0])
        tt(5, 4, 1, SL[1])
        tt(7, 6, 1, SL[1])
        sq(12, 6, SL[0], 32.0, -1.0)
        sq(12, 6, SL[1], 32.0, -1.0)
        sq(10, 5, SL[0], 4.0, 0.0)
        sq(10, 5, SL[1], 4.0, 0.0)
        tt(14, 7, 7, SL[1])
        tt(9, 8, 1, SL[0])
        # slab-0 leaves, per tc so the tc0 burst starts early
        for tcc in (0, 1):
            hs = (tcc * 32, (tcc + 1) * 32)
            tt(15, 14, 1, hs)
            tt(11, 10, 1, hs)
            tt(13, 12, 1, hs)
        tt(9, 8, 1, SL[1])
        for tcc in (2, 3):
            hs = (tcc * 32, (tcc + 1) * 32)
            tt(15, 14, 1, hs)
            tt(11, 10, 1, hs)
            tt(13, 12, 1, hs)

        # --- PE stream ---
        last_mm = None

        def junk(n):
            nonlocal last_mm
            for _ in range(n):
                mm = nc.tensor.matmul(
                    pjunk[:], warm[:], warm[:], start=True, stop=True,
                    skip_group_check=True,
                )
                if last_mm is not None:
                    add_dep_helper(mm.ins, last_mm.ins, sync=False,
                                   reason="junk chain")
                last_mm = mm

        junk(J1)
        for c in range(16):
            ub = uraw[:, c * 128 : (c + 1) * 128]
            mm = nc.tensor.matmul(
                psu[:], ub, ub, start=(c == 0), stop=(c == 15),
                skip_group_check=True,
            )
            add_dep_helper(mm.ins, last_mm.ins, sync=False, reason="ugram")
            last_mm = mm
        junk(J2)

        # Real bursts: per slab an opener matmul reads the slab's LAST
        # ACT-written feature (f10, per the pinned ACT order) so real
        # Ldweights carry only their single DVE wait.
        for si in (0, 1):
            b0, b1 = SL[si]
            fstub = FAke[:, 10, b1 - 1 : b1, 15:16]
            op = nc.tensor.matmul(
                pjunk[0:1, 120 + si : 121 + si], fstub, fstub,
                start=True, stop=True, skip_group_check=True,
            )
            add_dep_helper(op.ins, last_mm.ins, sync=False,
                           reason="opener after prev MMs")
            opener = op
            last_mm = op
            for tcc in (2 * si, 2 * si + 1):
                for b in range(BPC):
                    ps = pstiles[b]
                    for oc in range(8):
                        blk = (tcc * BPC + b) * 8 + oc
                        mm = nc.tensor.matmul(
                            ps[:], FAm[:, blk, 0], FAm[:, blk, 1],
                            start=(tcc == 0 and oc == 0),
                            stop=(tcc == TC - 1 and oc == 7),
                            skip_group_check=True,
                        )
                        add_dep_helper(mm.ins, opener.ins, sync=False,
                                       reason="PE wait-slot opener")
                        last_mm = mm

        # PSUM -> SBUF on ACT (psu early — it's done after the u-gram).
        _pin("act", nc.scalar.copy(
            out=csb[:, BPC * 128 : BPC * 128 + 128], in_=psu[:]))
        for b in range(BPC):
            _pin("act", nc.scalar.copy(
                out=csb[:, b * 128 : (b + 1) * 128], in_=pstiles[b][:]))
        nc.sync.dma_start(out=out_dram[:], in_=csb[:])


_CACHE = {}


def _build():
    if "nc" in _CACHE:
        return _CACHE["nc"]
    nc = bass.Bass("TRN2", debug=False)
    # Skip the kernel-tail DGE-queue drain (~3-4us); all DMAs are already
    # completion-waited by the split drains and no dynamic DMA state is
    # used.  Second-execution correctness is validated by the harness.
    type(nc.gpsimd).dma_reset = lambda self, semaphore_range=None: None
    xu_in = nc.dram_tensor("xu", [128, 2 * COLS], F16, kind="ExternalInput")
    pr_in = nc.dram_tensor("pr", [128, NPR], F32, kind="ExternalInput")
    out_d = nc.dram_tensor(
        "out", [128, BPC * 128 + 128 + NPROBE], F32, kind="ExternalOutput"
    )
    with TileContext(nc) as t:
        _body(nc, t, xu_in.ap(), pr_in.ap(), out_d.ap())
    _CACHE["nc"] = nc
    return nc


def _shard_host(a):
    """[T, B, N, D] -> per-core [128, COLS] fp16, col = tc*512+b*128+n*2+d."""
    out = []
    for c in range(N_CORES):
        s = a[:, c * BPC : (c + 1) * BPC]  # [512, 4, 64, 2]
        s = s.reshape(TC, 128, BPC, N_AGENTS, D)  # (tc, p, b, n, d)
        s = np.ascontiguousarray(np.transpose(s, (1, 0, 2, 3, 4)))
        out.append(s.reshape(128, COLS).astype(np.float16))
    return out


def _probe_args():
    xp = (np.arange(128, dtype=np.float64) + 0.5) / 128.0
    ks = np.arange(NPROBE, dtype=np.float64)
    a = np.empty((128, NPR), dtype=np.float32)
    a[:, :NPROBE] = (ks[None, :] * np.pi * xp[:, None] - np.pi / 2)
    a[:, NPROBE] = -np.pi / 2
    a[:, NPROBE + 1] = -1.0
    return a


def _make_inmaps(x, u):
    xs = _shard_host(np.asarray(x, dtype=np.float32))
    us = _shard_host(np.asarray(u, dtype=np.float32))
    prb = _probe_args()
    return [
        {
            "xu": np.ascontiguousarray(np.concatenate([xs[c], us[c]], axis=1)),
            "pr": prb,
        }
        for c in range(N_CORES)
    ]


def kernel(x, u, **_):
    nc = _build()
    in_maps = _make_inmaps(x, u)
    res = bass_utils.run_bass_kernel_spmd(nc, in_maps, core_ids=list(range(N_CORES)))
    return _finish_host(res.results)


_LAST_PROBE = None


def _finish_host(outs):
    """Host reduction/unmixing in float64 -> scalar loss."""
    global _LAST_PROBE
    Cp = np.zeros((B, K_MAX, K_MAX), dtype=np.float64)
    u2 = 0.0
    for c in range(N_CORES):
        o = outs[c]["out"].astype(np.float64)  # [128, 656]
        craw = o[:, : BPC * 128]
        ublk = o[:, BPC * 128 : BPC * 128 + 128]
        u2 += float(np.trace(ublk))
        for b in range(BPC):
            blk = craw[:, b * 128 : (b + 1) * 128]
            acc = np.zeros((K_MAX, K_MAX))
            for nl in range(8):
                acc += blk[nl::8, nl::8]
            Cp[c * BPC + b] = acc
    o0 = outs[0]["out"].astype(np.float64)
    dev = o0[:, BPC * 128 + 128 :]
    tru = np.sin(_probe_args()[:, :NPROBE].astype(np.float64))
    _LAST_PROBE = np.max(np.abs(dev - tru), axis=0)

    Ct = np.einsum("ik,bkl,jl->bij", _AINV, Cp, _AINV)
    c = Ct / (_NORM[None] * (N_AGENTS * T))
    loss = np.mean((c - _COEFFS[None]) ** 2)
    loss = loss + REG * u2 / (2.0 * N_AGENTS * T * B)
    return np.array(loss, dtype=np.float32)


if __name__ == "__main__":
    rng = np.random.default_rng(0)
    x = rng.random((T, B, N_AGENTS, D), dtype=np.float32)
    u = rng.standard_normal((T, B, N_AGENTS, D)).astype(np.float32)
    print(kernel(x=x, u=u))
    print("probe err per k:", _LAST_PROBE)
